# revision 2
# baseline (speedup 1.0000x reference)
"""PillarLayer scatter kernel for Trainium2 (8 NeuronCores, SPMD data-parallel).

Reference computation (per sample):
  center[p, :] = sum(pillars[p, :, :3], axis=1) / npoints[p]          # (40000, 3)
  canvas[x, y, :] = center  at coords (x, y)                          # scatter, unique cells
  out = canvas.transpose(2, 1, 0)                                     # (3, y_l, x_l)

Sharding: batch-parallel, one sample (40000 pillars) per NeuronCore.

Per-core device pipeline:
  1. Stream pillar chunks [128, T*128] f32, reduce points axis on DVE -> sums.
  2. Scale by 1/npoints (DVE), compute linear cell ids y*432+x from coors (DVE).
  3. Scatter 12-byte rows via indirect DMA, 128 rows/op (the only HW-supported
     form: one offset per partition), round-robined over 4 canvas tensors so
     the WAW chains pipeline. Pillar padding rows (host pads 40000->40064)
     carry cell id 214272 = a trash row outside the real canvas.
  4. Merge the 4 canvases + transpose pass: [cell, 3] -> out [3, cell].
Host reshapes out to (3, 496, 432) and stacks the 8 cores.
"""

import os

import numpy as np

P = 128
NP = 40000           # pillars per core (= per sample)
NPPAD = 40064        # padded to 128*313
XL, YL = 432, 496
NCELL = XL * YL      # 214272
NCANV = 4            # interleaved canvas tensors
BS = 8

LAST_RESULTS = None
LAST_EXEC_NS = None

_BUILT = None

# column counts per chunk: 12 chunks of 25 + 1 of 13  (sum = 313)
CHUNK_T = [25] * 12 + [13]


def build(nppad=NPPAD, chunk_t=None):
    import concourse.bacc as bacc
    import concourse.tile as tile
    import concourse.mybir as mybir
    from concourse.bass import IndirectOffsetOnAxis

    F32 = mybir.dt.float32
    I32 = mybir.dt.int32

    if chunk_t is None:
        chunk_t = CHUNK_T
    assert sum(chunk_t) * P == nppad

    nc = bacc.Bacc("TRN2", target_bir_lowering=False)

    pillars = nc.dram_tensor("pillars", [nppad, 128], F32, kind="ExternalInput")
    coors = nc.dram_tensor("coors", [nppad, 4], I32, kind="ExternalInput")
    npp = nc.dram_tensor("npp", [nppad], I32, kind="ExternalInput")
    canvs = [
        nc.dram_tensor(f"canv{i}", [NCELL + 1, 3], F32, kind="ExternalOutput")
        for i in range(NCANV)
    ]
    out = nc.dram_tensor("out", [3, NCELL], F32, kind="ExternalOutput")

    with tile.TileContext(nc) as tc:
        with (
            tc.tile_pool(name="pp", bufs=3) as ppool,
            tc.tile_pool(name="sp", bufs=3) as spool,
            tc.tile_pool(name="tp", bufs=2) as tpool,
        ):
            op_i = 0
            base = 0
            for T in chunk_t:
                n = P * T
                pt = ppool.tile([P, T * 128], F32, tag="pt")
                nc.sync.dma_start(
                    out=pt[:],
                    in_=pillars[base : base + n, :].rearrange(
                        "(p t) k -> p (t k)", p=P
                    ),
                )
                ct = spool.tile([P, T * 4], I32, tag="ct")
                nc.sync.dma_start(
                    out=ct[:],
                    in_=coors[base : base + n, :].rearrange(
                        "(p t) c -> p (t c)", p=P
                    ),
                )
                nt = spool.tile([P, T], I32, tag="nt")
                nc.sync.dma_start(
                    out=nt[:],
                    in_=npp[base : base + n].rearrange("(p t) -> p t", p=P),
                )

                # sum over the 32 points for each of the 4 channels
                sums = spool.tile([P, T * 4], F32, tag="sums")
                nc.vector.tensor_reduce(
                    out=sums[:].rearrange("p (t c) -> p t c", c=4),
                    in_=pt[:].rearrange("p (t pt c) -> p t c pt", pt=32, c=4),
                    axis=mybir.AxisListType.X,
                    op=mybir.AluOpType.add,
                )

                nf = spool.tile([P, T], F32, tag="nf")
                nc.vector.tensor_copy(out=nf[:], in_=nt[:])
                rinv = spool.tile([P, T], F32, tag="rinv")
                nc.vector.reciprocal(out=rinv[:], in_=nf[:])
                cent = spool.tile([P, T * 4], F32, tag="cent")
                nc.vector.tensor_tensor(
                    out=cent[:].rearrange("p (t c) -> p t c", c=4),
                    in0=sums[:].rearrange("p (t c) -> p t c", c=4),
                    in1=rinv[:].unsqueeze(2).to_broadcast([P, T, 4]),
                    op=mybir.AluOpType.mult,
                )

                # linear cell id = y*XL + x (x = coors col 1, y = col 2);
                # host-side padding rows carry (x=0, y=496) -> cell 214272 = trash row
                yf = spool.tile([P, T], F32, tag="yf")
                nc.vector.tensor_copy(
                    out=yf[:], in_=ct[:].rearrange("p (t c) -> p t c", c=4)[:, :, 2]
                )
                xf = spool.tile([P, T], F32, tag="xf")
                nc.vector.tensor_copy(
                    out=xf[:], in_=ct[:].rearrange("p (t c) -> p t c", c=4)[:, :, 1]
                )
                cellf = spool.tile([P, T], F32, tag="cellf")
                nc.vector.tensor_scalar_mul(out=cellf[:], in0=yf[:], scalar1=float(XL))
                nc.vector.tensor_tensor(
                    out=cellf[:], in0=cellf[:], in1=xf[:], op=mybir.AluOpType.add
                )
                offs = spool.tile([P, T], I32, tag="offs")
                nc.vector.tensor_copy(out=offs[:], in_=cellf[:])

                for t in range(T):
                    nc.gpsimd.indirect_dma_start(
                        out=canvs[op_i % NCANV][:],
                        out_offset=IndirectOffsetOnAxis(
                            ap=offs[:, t : t + 1], axis=0
                        ),
                        in_=cent[:, t * 4 : t * 4 + 3],
                        in_offset=None,
                    )
                    op_i += 1
                base += n

            tc.strict_bb_all_engine_barrier()

            # merge canvases + transpose: [cell, 3] -> out [3, cell]
            cell_base = 0
            for k in (512, 512, 512, 138):
                rts = []
                for i in range(NCANV):
                    rt = tpool.tile([P, 3 * k], F32, tag=f"rt{i}")
                    nc.sync.dma_start(
                        out=rt[:],
                        in_=canvs[i][cell_base : cell_base + P * k, :].rearrange(
                            "(p j) c -> p (j c)", p=P
                        ),
                    )
                    rts.append(rt)
                nc.vector.tensor_tensor(
                    out=rts[0][:], in0=rts[0][:], in1=rts[1][:],
                    op=mybir.AluOpType.add,
                )
                nc.vector.tensor_tensor(
                    out=rts[2][:], in0=rts[2][:], in1=rts[3][:],
                    op=mybir.AluOpType.add,
                )
                nc.vector.tensor_tensor(
                    out=rts[0][:], in0=rts[0][:], in1=rts[2][:],
                    op=mybir.AluOpType.add,
                )
                ut = tpool.tile([P, 3 * k], F32, tag="ut")
                for c, eng in ((0, nc.vector), (1, nc.gpsimd), (2, nc.vector)):
                    eng.tensor_copy(
                        out=ut[:, c * k : (c + 1) * k],
                        in_=rts[0][:].rearrange("p (j c) -> p c j", c=3)[:, c, :],
                    )
                nc.sync.dma_start(
                    out=out[:, cell_base : cell_base + P * k].rearrange(
                        "c (p j) -> p c j", p=P
                    ),
                    in_=ut[:].rearrange("p (c j) -> p c j", c=3),
                )
                cell_base += P * k
            assert cell_base == NCELL

    nc.compile()
    return nc


def _get_built():
    global _BUILT
    if _BUILT is None:
        _BUILT = build()
    return _BUILT


def _pad(arr, padval):
    pad_shape = (NPPAD - NP,) + arr.shape[1:]
    return np.concatenate([arr, np.full(pad_shape, padval, arr.dtype)], axis=0)


def kernel(pillars, coors_batch, npoints_per_pillar, bs=BS, x_l=XL, y_l=YL, **_):
    global LAST_RESULTS, LAST_EXEC_NS
    from concourse.bass_utils import run_bass_kernel_spmd

    nc = _get_built()

    pillars = np.asarray(pillars)
    coors_batch = np.asarray(coors_batch)
    npoints_per_pillar = np.asarray(npoints_per_pillar)

    p = np.ascontiguousarray(pillars, dtype=np.float32).reshape(BS, NP, 128)
    c = np.ascontiguousarray(coors_batch, dtype=np.int32).reshape(BS, NP, 4)
    n = np.ascontiguousarray(npoints_per_pillar, dtype=np.int32).reshape(BS, NP)

    # pad coors with (b=0, x=0, y=YL, z=0) -> cell id YL*XL = trash row
    pad_coor = np.zeros((NPPAD - NP, 4), np.int32)
    pad_coor[:, 2] = YL
    in_maps = []
    for i in range(BS):
        in_maps.append(
            {
                "pillars": _pad(p[i], 0.0),
                "coors": np.concatenate([c[i], pad_coor], axis=0),
                "npp": _pad(n[i], 1),
            }
        )
    trace = bool(int(os.environ.get("KBENCH_TRACE", "0")))
    res = run_bass_kernel_spmd(
        nc, in_maps, core_ids=list(range(BS)), trace=trace
    )
    LAST_RESULTS = res
    LAST_EXEC_NS = res.exec_time_ns

    outs = np.stack(
        [res.results[i]["out"].reshape(3, YL, XL) for i in range(BS)]
    )
    return (pillars, coors_batch, npoints_per_pillar, outs)
